# revision 78
# baseline (speedup 1.0000x reference)
"""Trainium2 Bass kernel for the EighMSE loss (data-parallel over 8 cores).

Math: row (a, b, c) encodes [[a, b], [b, c]];  SM = a+c, DF = a-c,
RT = sqrt(DF^2 + 4 b^2), x = clamp(DF/RT, -1, 1).
LAPACK larger-eval eigenvector g = (tau0*n0, tau1*n1u) with
n0 = sqrt((1+x)/2), n1u = sqrt((1-x)/2),
tau0 = -1 if DF > 0 else sign(b)*sign(SM);  tau1 = tau0*sign(b).

Per-core sums produced on device (per-partition partials, summed on host):
  SRT2 = sum RT2p + RT2t            (custom-op accum)
  Su   = sum (1+xp)(1+xt),  Sv = sum (1-xp)(1-xt)   -> Sx = (Su-Sv)/2
  D    = sum (bp-bt)^2              (custom-op accum)
  A    = sum (SMp-SMt)^2            (PE gram diag)
  C    = sum (DFp-DFt)^2            (PE gram diag)
  RTx  = sum RTp*RTt                (PE gram diag)
  Q0   = sum s0*P0,  Q1 = sum s1*P1,  Qm0 = sum s0*sxpm*P0,  Qm1 (PE grams)
where P0 = n0p*n0t = sqrt(u)/2, P1 = n1up*n1ut = sqrt(v)/2,
s0 = tau0p*tau0t = psi_p*psi_t (psi = select(DF>0, -1, sign(b)sign(SM))),
s1 = s0*sign(bp)sign(bt), sxpm = sign(SMp)*sign(SMt).
Host combine:
  Bs = SRT2 - 2*RTx;  E1 = B + Sx/2 - 2*Q0;  E2 = B - Sx/2 - 2*Q1
  F0 = E1 + 2*(Q0-Qm0);  F1 = E2 + 2*(Q1-Qm1)
  loss = w0*(A+Bs)/(4B) + w1*E1/B + w2*E2/B + w3*F1/B + w4*F0/B
         + w5*(A/2 + C/2 + D)/(3B)
"""

import operator
import numpy as np
from contextlib import ExitStack

import concourse.bass as bass
import concourse.bacc as bacc
import concourse.tile as tile
from concourse import mybir
from concourse.bass_utils import run_bass_kernel_spmd
import concourse.dve_ops as dve_ops
from concourse.dve_spec import (
    Spec, Src0, Src1, C0, C1, Zero, One, AluOp, Bin,
    sq, maxx, minn, select, lower, _has_src1 as has_src1,
)
from concourse.dve_uop import DveOpSpec

F32 = mybir.dt.float32
BF16 = mybir.dt.bfloat16
I32 = mybir.dt.int32
OP = mybir.AluOpType
AF = mybir.ActivationFunctionType

B_TOTAL = 4_194_304
NCORES = 8
S = B_TOTAL // NCORES          # samples per core
P = 128                        # partitions
NPC = S // P                   # samples per partition (4096)
W = 1024                       # samples per tile per partition (per tensor)
NT = NPC // W                  # tiles per core
W2 = 2 * W
NSTAT = 27


def _register(name, spec, subdim=False):
    if name in dve_ops._SUB_OPCODE_FOR_NAME:
        for op in dve_ops.OPS:
            if op.name == name:
                return op
    row = dve_ops._CUSTOM_DVE_ROW_BASE + len(dve_ops.OPS)
    shas = {}
    for ver in ("v3", "v4"):
        uops = lower(spec, ver=ver)
        shas[ver] = DveOpSpec(name=name, opcode=row, uops=uops,
                              rd1_en=has_src1(spec)).sha(ver)
    op = dve_ops.DveOp(name, spec, subdim=subdim, uops_sha=shas)
    dve_ops.OPS.append(op)
    dve_ops.CUSTOM_DVE_SPECS[name] = spec
    dve_ops._SUB_OPCODE_FOR_NAME[name] = row
    return op


def _acc_ref(body):
    def _r(in0, in1, c0, c1, c2):
        b = body(in0.astype(np.float32),
                 None if in1 is None else np.asarray(in1, np.float32),
                 c0, c1, c2).astype(np.float32)
        return b, c0 + b.reshape(b.shape[0], -1).sum(axis=-1, keepdims=True)
    return _r


_negone = Zero - One
_clamp0 = minn(maxx(Src0, _negone), One)
_clamp1 = minn(maxx(Src1, _negone), One)

RT2_OP = _register("EIGH_RT2_ANT", Spec(
    body=sq(Src0) + sq(Src1 + Src1) + C1,
    accum=operator.add, accum_init=C0,
    reference=_acc_ref(lambda i0, i1, c0, c1, c2: i0 * i0 + 4.0 * i1 * i1 + c1)))

U_OP = _register("EIGH_U_ANT", Spec(
    body=(One + Src0) * (One + Src1),
    accum=operator.add, accum_init=C0,
    reference=_acc_ref(lambda i0, i1, c0, c1, c2: (1.0 + i0) * (1.0 + i1))))

V_OP = _register("EIGH_V_ANT", Spec(
    body=(One - Src0) * (One - Src1),
    accum=operator.add, accum_init=C0,
    reference=_acc_ref(lambda i0, i1, c0, c1, c2: (1.0 - i0) * (1.0 - i1))))

_nx = Bin(AluOp.BITWISE_NOT, Src0, Src0)
_y0 = _nx * C0
_y1 = _y0 * (C1 - Src0 * _y0)
_RC0 = -0.23549792
_RC1 = 2.0017324


def _ref_xdiv(in0, in1, s0, s1, imm2):
    nx = (~in0.astype(np.float32).view(np.int32)).view(np.float32)
    y0 = nx * s0
    y1 = y0 * (s1 - in0.astype(np.float32) * y0)
    return np.clip(y1 * in1, -1.0, 1.0).astype(np.float32)


X_OP = _register("EIGH_XDIV_ANT", Spec(
    body=minn(maxx(_y1 * Src1, _negone), One),
    reference=_ref_xdiv))

PSI_OP = _register("EIGH_PSI_ANT", Spec(
    body=select(Src0 > Zero, _negone, Src1),
    reference=lambda in0, in1, s0, s1, imm2:
        np.where(in0 > 0, np.float32(-1.0), in1).astype(np.float32)))

DSQ_OP = _register("EIGH_DSQ_ANT", Spec(
    body=sq(Src0 - Src1),
    accum=operator.add, accum_init=C0,
    reference=_acc_ref(lambda i0, i1, c0, c1, c2: (i0 - i1) ** 2)))

_BUILT = None


def _build_bass():
    nc = bacc.Bacc()
    yp = nc.declare_dram_parameter("y_pred", [S, 3], F32, isOutput=False)
    yt = nc.declare_dram_parameter("y_true", [S, 3], F32, isOutput=False)
    out = nc.declare_dram_parameter("out", [P, NSTAT], F32, isOutput=True)

    ypr = yp.rearrange("(p n) c -> p n c", p=P)
    ytr = yt.rearrange("(p n) c -> p n c", p=P)

    with tile.TileContext(nc) as tc, ExitStack() as ctx:
        inp = ctx.enter_context(tc.tile_pool(name="inp", bufs=2))
        wk = ctx.enter_context(tc.tile_pool(name="wk", bufs=2))
        wq = ctx.enter_context(tc.tile_pool(name="wq", bufs=3))
        cst = ctx.enter_context(tc.tile_pool(name="cst", bufs=1))
        psA = ctx.enter_context(tc.tile_pool(name="psA", bufs=1, space="PSUM"))

        stats = cst.tile([P, NSTAT], F32)
        nc.vector.memset(stats[:], 0.0)
        tiny = cst.tile([P, 1], F32)
        nc.vector.memset(tiny[:], 1e-30)
        ii = cst.tile([P, 128], I32)
        nc.gpsimd.iota(ii[:], pattern=[[1, 128]], channel_multiplier=-1)
        I128 = cst.tile([P, 128], BF16)
        nc.vector.tensor_single_scalar(I128[:], ii[:], 0, op=OP.is_equal)

        regA = psA.tile([P, 512], F32)   # SMself, SMcross, DFself, DFcross
        regB = psA.tile([P, 512], F32)   # RTx, Q0, Q1, Qm0
        regC = psA.tile([P, 256], F32)   # Qm1, Dself
        REGIONS = [(regA, 0), (regA, 1), (regA, 2), (regA, 3),
                   (regB, 0), (regB, 1), (regB, 2), (regB, 3),
                   (regC, 0), (regC, 1)]

        def phase1(i):
            X = inp.tile([P, W2, 3], F32, tag="X")
            nc.sync.dma_start(X[:, 0:W, :], ypr[:, bass.ts(i, W), :])
            nc.sync.dma_start(X[:, W:W2, :], ytr[:, bass.ts(i, W), :])
            a = X[:, :, 0]
            bcol = X[:, :, 1]
            c = X[:, :, 2]

            DF = wk.tile([P, W2], BF16, tag="DF")
            SM = wk.tile([P, W2], BF16, tag="SM")
            sgb = wk.tile([P, W2], BF16, tag="sgb")
            sgs = wk.tile([P, W2], BF16, tag="sgs")
            RT2 = wk.tile([P, W2], BF16, tag="RT2")
            RT = wk.tile([P, W2], BF16, tag="RT")
            if i == 0:
                for h, col in ((slice(0, W), 0), (slice(W, W2), 26)):
                    nc.vector.tensor_sub(DF[:, h], X[:, h, 0], X[:, h, 2])
                    nc.vector.tensor_add(SM[:, h], X[:, h, 0], X[:, h, 2])
                    nc.scalar.activation(sgb[:, h], X[:, h, 1], AF.Sign,
                                         bias=tiny[:])
                    nc.scalar.activation(sgs[:, h], SM[:, h], AF.Sign,
                                         bias=tiny[:])
                    nc.vector._custom_dve(RT2_OP, out=RT2[:, h], in0=DF[:, h],
                                          in1=X[:, h, 1], s0=0.0, s1=1e-20,
                                          accum_out=stats[:, col:col + 1])
                    nc.scalar.activation(RT[:, h], RT2[:, h], AF.Sqrt)
            else:
                for h in (slice(0, W), slice(W, W2)):
                    nc.gpsimd.tensor_sub(DF[:, h], X[:, h, 0], X[:, h, 2])
                for h in (slice(0, W), slice(W, W2)):
                    nc.gpsimd.tensor_add(SM[:, h], X[:, h, 0], X[:, h, 2])
                nc.scalar.activation(sgb[:], bcol, AF.Sign, bias=tiny[:])
                nc.scalar.activation(sgs[:], SM[:], AF.Sign, bias=tiny[:])
                nc.vector._custom_dve(RT2_OP, out=RT2[:], in0=DF[:], in1=bcol,
                                      s0=0.0, s1=1e-20,
                                      accum_out=stats[:, 0 + i:1 + i])
                nc.scalar.activation(RT[:], RT2[:], AF.Sqrt)
            pre = wk.tile([P, W2], BF16, tag="pre")
            nc.vector.tensor_mul(pre[:], sgb[:], sgs[:])
            db = wq.tile([P, W], BF16, tag="db")
            nc.gpsimd.tensor_sub(db[:], X[:, 0:W, 1], X[:, W:W2, 1])
            return dict(X=X, DF=DF, SM=SM, sgb=sgb, sgs=sgs, RT=RT, pre=pre,
                        db=db)

        def phase2(i, T):
            X, DF, SM = T["X"], T["DF"], T["SM"]
            sgb, sgs, RT, pre = T["sgb"], T["sgs"], T["RT"], T["pre"]
            x = wk.tile([P, W2], BF16, tag="x")
            nc.vector._custom_dve(X_OP, out=x[:], in0=RT[:], in1=DF[:],
                                  s0=_RC0, s1=_RC1)

            u = wq.tile([P, W], BF16, tag="u")
            nc.vector._custom_dve(U_OP, out=u[:], in0=x[:, 0:W], in1=x[:, W:W2],
                                  s0=0.0, accum_out=stats[:, 4 + i:5 + i])
            v = wq.tile([P, W], BF16, tag="v")
            nc.vector._custom_dve(V_OP, out=v[:], in0=x[:, 0:W], in1=x[:, W:W2],
                                  s0=0.0, accum_out=stats[:, 8 + i:9 + i])
            P0 = wq.tile([P, W], BF16, tag="P0")
            nc.scalar.activation(P0[:], u[:], AF.Sqrt, scale=0.25)
            P1 = wq.tile([P, W], BF16, tag="P1")
            nc.scalar.activation(P1[:], v[:], AF.Sqrt, scale=0.25)

            psi = wk.tile([P, W2], BF16, tag="psi")
            nc.vector._custom_dve(PSI_OP, out=psi[:], in0=DF[:], in1=pre[:])

            s0 = wq.tile([P, W], BF16, tag="s0")
            nc.vector.tensor_mul(s0[:], psi[:, 0:W], psi[:, W:W2])
            sbx = wq.tile([P, W], BF16, tag="sbx")
            nc.gpsimd.tensor_mul(sbx[:], sgb[:, 0:W], sgb[:, W:W2])
            s1 = wq.tile([P, W], BF16, tag="s1")
            nc.vector.tensor_mul(s1[:], s0[:], sbx[:])
            sxp = wq.tile([P, W], BF16, tag="sxp")
            nc.vector.tensor_mul(sxp[:], sgs[:, 0:W], sgs[:, W:W2])
            s0m = wq.tile([P, W], BF16, tag="s0m")
            nc.vector.tensor_mul(s0m[:], s0[:], sxp[:])
            s1m = wq.tile([P, W], BF16, tag="s1m")
            nc.vector.tensor_mul(s1m[:], s1[:], sxp[:])

            db = T["db"]

            # (k, lhs_tile, lhs_base, rhs_tile, rhs_base, nhalves)
            grams = [(0, SM, 0, SM, 0), (0, SM, W, SM, W),
                     (1, SM, 0, SM, W),
                     (2, DF, 0, DF, 0), (2, DF, W, DF, W),
                     (3, DF, 0, DF, W),
                     (4, RT, 0, RT, W),
                     (9, db, 0, db, 0),
                     (5, s0, 0, P0, 0), (6, s1, 0, P1, 0),
                     (7, s0m, 0, P0, 0), (8, s1m, 0, P1, 0)]
            nsub = {}
            for k, *_ in grams:
                nsub[k] = nsub.get(k, 0) + 1
            seen = {}
            # start=True zeroes the whole 2KB psum bank, so only the first
            # matmul of each BANK (k = 0, 4, 8) may carry it.  Region-major
            # order lets early regions stop sooner so extraction can hoist.
            for k, lt, lb, rt_, rb in grams:
                for c8 in range(8):
                    lo = c8 * 128
                    seen[k] = seen.get(k, 0) + 1
                    st = (i == 0 and seen[k] == 1 and k in (0, 4, 9))
                    sp = (i == NT - 1 and seen[k] == nsub[k] * 8)
                    reg, off = REGIONS[k]
                    nc.tensor.matmul(
                        reg[:, off * 128:(off + 1) * 128],
                        lt[:, lb + lo:lb + lo + 128],
                        rt_[:, rb + lo:rb + lo + 128], start=st, stop=sp)

        for i in range(NT):
            phase2(i, phase1(i))

        Irep = cst.tile([P, 512], BF16)
        for r in range(4):
            nc.vector.tensor_copy(Irep[:, r * 128:(r + 1) * 128], I128[:])
        scr = cst.tile([P, 512], F32)
        nc.vector.tensor_mul(scr[:], regA[:], Irep[:])
        nc.vector.tensor_reduce(
            stats[:, 16:20], scr[:].rearrange("p (r c) -> p r c", c=128),
            axis=mybir.AxisListType.X, op=OP.add)
        nc.vector.tensor_mul(scr[:], regB[:], Irep[:])
        nc.vector.tensor_reduce(
            stats[:, 20:24], scr[:].rearrange("p (r c) -> p r c", c=128),
            axis=mybir.AxisListType.X, op=OP.add)
        nc.vector.tensor_mul(scr[:, 0:256], regC[:], Irep[:, 0:256])
        nc.vector.tensor_reduce(
            stats[:, 24:26], scr[:, 0:256].rearrange("p (r c) -> p r c", c=128),
            axis=mybir.AxisListType.X, op=OP.add)

        nc.sync.dma_start(out[:, :], stats[:])

    nc.compile()
    return nc


def _get_built():
    global _BUILT
    if _BUILT is None:
        _BUILT = _build_bass()
    return _BUILT


def kernel(y_pred: np.ndarray, y_true: np.ndarray, weights: np.ndarray) -> np.ndarray:
    y_pred = np.ascontiguousarray(y_pred, dtype=np.float32)
    y_true = np.ascontiguousarray(y_true, dtype=np.float32)
    w = np.asarray(weights, dtype=np.float64)

    nc = _get_built()
    in_maps = []
    for cix in range(NCORES):
        in_maps.append({
            "y_pred": y_pred[cix * S:(cix + 1) * S],
            "y_true": y_true[cix * S:(cix + 1) * S],
        })
    res = run_bass_kernel_spmd(nc, in_maps, list(range(NCORES)))
    sums = np.zeros(NSTAT, dtype=np.float64)
    for cix in range(NCORES):
        sums += np.asarray(res.results[cix]["out"], dtype=np.float64).sum(axis=0)

    SRT2 = sums[0:4].sum() + sums[26]
    Sx = (sums[4:8].sum() - sums[8:12].sum()) / 2.0

    SMself, SMx, DFself, DFx, RTx, Q0, Q1, Qm0 = sums[16:24]
    Qm1 = sums[24]
    D = sums[25]
    A = SMself - 2.0 * SMx
    C = DFself - 2.0 * DFx

    Bn = float(B_TOTAL)
    Bs = SRT2 - 2.0 * RTx
    E1 = Bn + Sx / 2.0 - 2.0 * Q0
    E2 = Bn - Sx / 2.0 - 2.0 * Q1
    F0 = E1 + 2.0 * (Q0 - Qm0)
    F1 = E2 + 2.0 * (Q1 - Qm1)
    evals_mse = (A + Bs) / (4.0 * Bn)
    mse_loss = (0.5 * A + 0.5 * C + D) / (3.0 * Bn)
    loss = (w[0] * evals_mse + w[1] * E1 / Bn + w[2] * E2 / Bn
            + w[3] * F1 / Bn + w[4] * F0 / Bn + w[5] * mse_loss)
    return np.float32(loss)
